# revision 34
# baseline (speedup 1.0000x reference)
"""Causal self-attention (B=2, N=2048, D=1024, H=16, hd=64) on 8 trn2 NeuronCores.

Sharding: core c handles batch b = c//4 and 4 heads hs = [4*(c%4) .. 4*(c%4)+3]
(tensor-parallel over heads x data-parallel over batch). Each core computes its
heads' attention and a row-parallel partial of the output projection
(partial[n, :] = sum_{local heads} sa_h[n, :] @ Wo[h*64:(h+1)*64, :]); the host
sums the 4 partials per batch and adds the output bias.

Device schedule (v3, ~132us cost-model vs 164us baseline):
  - x/Wqkv inputs stream in bf16 (halves DMA bytes; rel err ~4e-3 vs 2e-2
    gate); q/k/v, scores and attn@v stay fp32r. Output partials in bf16.
  - One DMA instruction per tensor half (HWDGE costs ~625ns per instruction,
    so few large DMAs; wv/xT block-0 halves land first and gate the first
    matmuls at ~4us).
  - 8 dummy rank-1 matmuls ramp the PE p-state (0.65->2.4GHz needs 3us of
    continuous execution) while the first input DMAs are in flight.
  - Startup computes v(0..3)+qk(0) with per-kt partial accumulation in 8
    concurrent PSUM banks, emission ordered to match DMA arrival.
  - attention(J) emits per-(pair, key-tile) units: 2 score matmuls (head pair
    stacked on partition halves, K=64 each), one fused Exp on Act
    (scale=1/8 folded), causal tri-mask on DVE for diagonal tiles, and the
    attn@v accumulation lagging 3 units so the Exp latency hides behind later
    score matmuls. The d=3 diagonal matmuls run 256 wide (fp32r below 256
    costs 4x) with a [zeros|tri] mask zeroing the extra columns.
  - v65 tiles carry a ones-column so attn@v row 64 accumulates the softmax
    denominator for free; normalization = DVE reciprocal -> gpsimd partition
    broadcast -> DVE multiply into saT.
  - v/qk/out-projection "filler" pieces are deliberately NOT woven between
    attention units (that starves the Act pipeline); they clump at pair-flush
    and between-attention boundaries where the PE has genuine dead time
    (fpad=1000 disables mid-unit spreading).
  - Endgame: block-3 out-projection pieces spread over all 8 PSUM banks
    (sc/acc tags are retired by then); their kt2=0 halves run inside the
    final normalize latency window; copies split across Act and DVE; one
    output DMA per 128-row block.
  - PSUM bank discipline: one accumulation group per 2KB bank (start=True
    re-zeroes the bank-granular zero region, so groups must not share banks).
"""

import numpy as np
import ml_dtypes
from contextlib import ExitStack

import concourse.bass as bass
import concourse.tile as tile
from concourse import bacc, mybir
from concourse import bass_utils

F32 = mybir.dt.float32
F32R = mybir.dt.float32r
BF16 = mybir.dt.bfloat16
EXP = mybir.ActivationFunctionType.Exp

B, N, D, H, HD = 2, 2048, 1024, 16, 64
N_CORES = 8
LH = 4            # local heads per core
KT = D // 128     # 8 contraction k-tiles
NT = N // 128     # 16 n-tiles
NB = N // 512     # 4 n-blocks / q-blocks
QB = 512

_CACHE: dict = {}

CFG = {
    "at_bufs": 6,
    "oe_bufs": 4,
    "sc_bufs": 2,
    "acc_lag": 3,
    "act_primer": True,
    "n_warmup": 8,
    "fpad": 1000,
    "mask_pool": False,
    "flush_fill": 2,
    "exp_strided": False,
    "dump_split": False,
}


def _emit(nc, tc, ctx, io, repeat=1, dbg=None):
    xT, wqk, wv, bqk, bv, wo, tri2, out = io

    persist = ctx.enter_context(tc.tile_pool(name="persist", bufs=1))
    sbp = ctx.enter_context(tc.tile_pool(name="work", bufs=1))
    psum = ctx.enter_context(tc.tile_pool(name="psum", bufs=1, space="PSUM"))

    # ---- persistent SBUF tensors ----
    xT_sb = persist.tile([128, KT, N], BF16)
    wqk_sb = persist.tile([128, KT, 512], BF16)
    wv_sb = persist.tile([128, KT, 256], BF16)
    wo_sb = persist.tile([128, 2, 1024], F32R)
    bqk_sb = persist.tile([128, 4], F32)
    bv_sb = persist.tile([1, 256], F32)
    bvbc_sb = persist.tile([128, 256], F32)
    ones_sb = persist.tile([1, 128], F32)
    tri2_sb = persist.tile([128, 256], F32R)
    qkT_sb = persist.tile([128, 4, N], F32R)
    v65_sb = persist.tile([128, NT, LH * 65], F32R)
    saT_sb = persist.tile([128, 2, N], F32R)

    # ---- PE warmup + constants first (independent of any DMA) ----
    nc.vector.memset(ones_sb[:], 1.0)
    if CFG["n_warmup"]:
        # ramp the PE p-state while the first input chunks stream in: dummy
        # rank-1 matmuls on a zeroed row into a never-read psum bank
        zz = persist.tile([1, 512], F32R)
        nc.vector.memset(zz[:].bitcast(F32), 0.0)
        wps = psum.tile([128, 512], F32, name="ps_op", tag="op", bufs=2)
        for i in range(CFG["n_warmup"]):
            nc.tensor.matmul(
                wps[:], ones_sb[0:1, 0:128].bitcast(F32R), zz[:],
                start=(i == 0), stop=(i == CFG["n_warmup"] - 1),
                skip_group_check=True,
            )
    # the +1 denominator row of v65: free index h*65+64 per (nt, h)
    ones_col = v65_sb[:, :, :].rearrange("p n (h c) -> p n h c", c=65)[:, :, :, 64:65]
    nc.vector.memset(ones_col.bitcast(F32), 1.0)
    if CFG["act_primer"]:
        primer = sbp.tile([1, 1], F32, name="t_primer", tag="primer", bufs=1)
        nc.scalar.activation(primer[:], ones_sb[0:1, 0:1], EXP)

    # ---- input DMAs (SP issue order == DMA service order; HWDGE costs
    # ~625ns per instruction, so one instruction per tensor block) ----
    for h in range(2):
        k0, k1 = h * 512, (h + 1) * 512
        nc.sync.dma_start(
            wv_sb[:, 4 * h:4 * h + 4, :],
            wv[k0:k1, :].rearrange("(kt p) c -> p kt c", p=128))
        nc.sync.dma_start(
            xT_sb[:, 4 * h:4 * h + 4, 0:QB],
            xT[k0:k1, 0:QB].rearrange("(kt p) c -> p kt c", p=128))
    for h in range(2):
        k0, k1 = h * 512, (h + 1) * 512
        nc.sync.dma_start(
            wqk_sb[:, 4 * h:4 * h + 4, :],
            wqk[k0:k1, :].rearrange("(kt p) c -> p kt c", p=128))
    nc.sync.dma_start(bqk_sb[:], bqk.rearrange("t p -> p t"))
    nc.sync.dma_start(bv_sb[:], bv[:])
    nc.sync.dma_start(tri2_sb[:], tri2[:])
    for nb in range(1, NB):
        nc.sync.dma_start(
            xT_sb[:, :, nb * QB:(nb + 1) * QB],
            xT[:, nb * QB:(nb + 1) * QB].rearrange("(kt p) c -> p kt c", p=128),
        )
    for kt2 in range(2):
        nc.sync.dma_start(wo_sb[:, kt2, :], wo[kt2 * 128:(kt2 + 1) * 128, :])
    # broadcast v-bias across partitions once (Pool is idle)
    nc.gpsimd.partition_broadcast(bvbc_sb[:], bv_sb[:])

    # ---- phase pieces ----
    def v_mm(ps, nt, kt):
        # one 256-wide v matmul; a psum bank holds exactly one group
        # (start=True re-zeroes the whole 2KB zero region, so banks can't be
        # shared between accumulation groups)
        nc.tensor.matmul(
            ps[:, 0:256],
            xT_sb[:, kt, nt * 128:(nt + 1) * 128],
            wv_sb[:, kt, :],
            start=(kt == 0), stop=(kt == KT - 1),
            skip_group_check=True,
        )

    def v_drain(ps, nt):
        # psum -> v65 with the v-bias folded in (bias varies along free dim)
        dst = v65_sb[:, nt, :].rearrange("p (h c) -> p h c", c=65)[:, :, 0:64]
        src = ps[:, 0:256].rearrange("p (h c) -> p h c", c=64)
        bia = bvbc_sb[:, :].rearrange("p (h c) -> p h c", c=64)
        nc.vector.tensor_add(dst, src, bia)

    def qk_ct_mm(ps, nb, ct, kts):
        for kt in kts:
            nc.tensor.matmul(
                ps[:], wqk_sb[:, kt, ct * 128:(ct + 1) * 128],
                xT_sb[:, kt, nb * QB:(nb + 1) * QB],
                start=(kt == 0), stop=(kt == KT - 1),
                skip_group_check=True,
            )

    def qk_ct_drain(ps, nb, ct, eng="vector"):
        if eng == "scalar":
            nc.scalar.activation(
                qkT_sb[:, ct, nb * QB:(nb + 1) * QB], ps[:],
                mybir.ActivationFunctionType.Identity,
                bias=bqk_sb[:, ct:ct + 1])
        else:
            nc.vector.tensor_scalar_add(
                qkT_sb[:, ct, nb * QB:(nb + 1) * QB], ps[:], bqk_sb[:, ct:ct + 1])

    # steady-state filler pieces (each ~0.4-1 us of PE work)
    def fill_v(nt):
        ps = psum.tile([128, 512], F32, name="ps_op", tag="op", bufs=2)
        def h1():
            for kt in range(4):
                v_mm(ps, nt, kt)
        def h2():
            for kt in range(4, KT):
                v_mm(ps, nt, kt)
            v_drain(ps, nt)
        return [h1, h2]

    def fill_qk_ct(nb, ct):
        ps = psum.tile([128, 512], F32, name="ps_op", tag="op", bufs=2)
        def h1():
            qk_ct_mm(ps, nb, ct, range(4))
        def h2():
            qk_ct_mm(ps, nb, ct, range(4, KT))
            # Act has slack during att(0)/att(1); DVE is the tighter engine
            qk_ct_drain(ps, nb, ct, eng="scalar" if nb <= 2 else "vector")
        return [h1, h2]

    def fill_op(J, nqs, copy_eng="vector", dma_per_dh=False):
        # one output-projection row-block piece: both dh halves -> one DMA
        r0 = J * QB + nqs * 128
        oe = sbp.tile([128, 1024], BF16, name="t_oe", tag="oe", bufs=CFG["oe_bufs"])
        def piece(dh):
            def f():
                op = psum.tile([128, 512], F32, name="ps_op", tag="op", bufs=2)
                for kt2 in range(2):
                    nc.tensor.matmul(
                        op[:], saT_sb[:, kt2, r0:r0 + 128],
                        wo_sb[:, kt2, dh * 512:(dh + 1) * 512],
                        start=(kt2 == 0), stop=(kt2 == 1),
                        skip_group_check=True,
                    )
                if copy_eng == "scalar":
                    nc.scalar.copy(oe[:, dh * 512:(dh + 1) * 512], op[:])
                else:
                    nc.vector.tensor_copy(oe[:, dh * 512:(dh + 1) * 512], op[:])
                if dma_per_dh:
                    nc.sync.dma_start(out[r0:r0 + 128, dh * 512:(dh + 1) * 512],
                                      oe[:, dh * 512:(dh + 1) * 512])
                elif dh == 1:
                    nc.sync.dma_start(out[r0:r0 + 128, :], oe[:])
            return f
        return [piece(0), piece(1)]

    # ---- attention ----
    def emit_acc(J, accs, unit, n_t):
        t, p, at, c0e = unit
        for s in range(2):
            nc.tensor.matmul(
                accs[p][s][0:65, c0e:512],
                v65_sb[:, t, (2 * p + s) * 65:(2 * p + s) * 65 + 65],
                at[:, s * 512 + c0e:(s + 1) * 512],
                start=(t == 0), stop=(t == n_t - 1),
                skip_group_check=True,
            )

    def normalize(J, accs, p, split=False):
        halves = ((0, 256), (256, 512)) if split else ((0, 512),)
        for a, b in halves:
            bcs = []
            for s in range(2):
                rc = sbp.tile([1, 512], F32, name="t_rc", tag="rc", bufs=4)
                nc.vector.reciprocal(rc[:, 0:b - a], accs[p][s][64:65, a:b])
                bc = sbp.tile([64, 512], F32, name="t_bc", tag="bc", bufs=4)
                nc.gpsimd.partition_broadcast(bc[:, 0:b - a], rc[:, 0:b - a])
                bcs.append(bc)
            for s in range(2):
                nc.vector.tensor_mul(
                    saT_sb[s * 64:(s + 1) * 64, p, J * QB + a:J * QB + b],
                    accs[p][s][0:64, a:b], bcs[s][:, 0:b - a],
                )

    def attention(J, fillers):
        n_t = 4 * J + 4
        n_units = n_t * 2
        n_fill = len(fillers)
        fill_i = 0
        unit_i = 0
        for p in range(2):
            accs = {p: [psum.tile([128, 512], F32, name="ps_acc", tag="acc",
                                  bufs=2) for _ in range(2)]}
            pend = []
            for t in range(n_t):
                d = t - 4 * J
                c0 = max(d, 0) * 128
                c0e = min(c0, 256)
                sc = psum.tile([128, 1024], F32, name="ps_sc", tag="sc",
                               bufs=CFG["sc_bufs"])
                for s in range(2):
                    nc.tensor.matmul(
                        sc[:, s * 512 + c0e:(s + 1) * 512],
                        qkT_sb[s * 64:(s + 1) * 64, 2 * p, t * 128:(t + 1) * 128],
                        qkT_sb[s * 64:(s + 1) * 64, 2 * p + 1, J * QB + c0e:(J + 1) * QB],
                        start=True, stop=True,
                    )
                at = sbp.tile([128, 1024], F32R, name="t_at", tag="at",
                              bufs=CFG["at_bufs"])
                if CFG["exp_strided"] and c0e > 0:
                    atv3 = at[:, :].rearrange(
                        "p (s w) -> p s w", s=2)[:, :, c0e:512]
                    scv3 = sc[:, :].rearrange(
                        "p (s w) -> p s w", s=2)[:, :, c0e:512]
                    nc.scalar.activation(atv3, scv3, EXP, scale=0.125)
                else:
                    nc.scalar.activation(at[:, c0e:1024], sc[:, c0e:1024],
                                         EXP, scale=0.125)
                mul_eng = nc.gpsimd if CFG["mask_pool"] else nc.vector
                if d == 3:
                    atv = at[:, 256:1024].rearrange(
                        "p (s c) -> p s c", c=256)[:, ::2, :]
                    mul_eng.tensor_mul(
                        atv, atv, tri2_sb[:, None, :].broadcast_to([128, 2, 256]))
                elif d >= 0:
                    atv = at[:, c0:c0 + 640].rearrange(
                        "p (s c) -> p s c", c=128)[:, ::4, :]
                    mul_eng.tensor_mul(
                        atv, atv,
                        tri2_sb[:, None, 128:256].broadcast_to([128, 2, 128]))
                pend.append((t, p, at, c0e))
                if len(pend) > CFG["acc_lag"]:
                    emit_acc(J, accs, pend.pop(0), n_t)
                unit_i += 1
                # proportional filler spreading (padded so some remain for the
                # pair-flush stalls)
                while fill_i < n_fill and fill_i * (n_units + CFG["fpad"]) < unit_i * n_fill:
                    fillers[fill_i]()
                    fill_i += 1
            # flush the lag: earlier accs are already unblocked - emit them
            # first, then cover the last Exp's latency with fillers
            while pend:
                emit_acc(J, accs, pend.pop(0), n_t)
                if len(pend) == 1:
                    for _ in range(CFG["flush_fill"]):
                        if fill_i < n_fill:
                            fillers[fill_i]()
                            fill_i += 1
            normalize(J, accs, p, split=(J == 3 and p == 1))
            # dump a share of the remaining fillers at this pair boundary
            if CFG["dump_split"] and p == 0:
                target = fill_i + (n_fill - fill_i) // 2
                while fill_i < target:
                    fillers[fill_i]()
                    fill_i += 1
        while fill_i < n_fill:
            fillers[fill_i]()
            fill_i += 1

    # ---- emission ----
    for _rep in range(repeat):
        # startup: kt-partial v(0..3) + qk(0), 8 concurrent psum banks
        v_ps = ([psum.tile([128, 512], F32, name="ps_acc", tag="acc", bufs=2)
                 for _ in range(2)]
                + [psum.tile([128, 512], F32, name="ps_op", tag="op", bufs=2)
                   for _ in range(2)])
        qk_ps = [psum.tile([128, 1024], F32, name="ps_sc", tag="sc",
                           bufs=CFG["sc_bufs"]) for _ in range(2)]
        # emission matches DMA arrival: wv/xT halves land before wqk halves
        for kh in range(2):
            for kt in range(4 * kh, 4 * kh + 4):
                for i in range(4):
                    v_mm(v_ps[i], i, kt)
        for kh in range(2):
            for kt in range(4 * kh, 4 * kh + 4):
                for ct in range(4):
                    qk_ct_mm(qk_ps[ct // 2][:, (ct % 2) * 512:(ct % 2 + 1) * 512],
                             0, ct, [kt])
        for ct in range(4):
            qk_ct_drain(qk_ps[ct // 2][:, (ct % 2) * 512:(ct % 2 + 1) * 512],
                        0, ct, eng="scalar")
        for i in range(4):
            v_drain(v_ps[i], i)

        attention(0, fill_v(4) + fill_v(5) + fill_v(6) + fill_v(7)
                  + fill_qk_ct(1, 0) + fill_qk_ct(1, 1)
                  + fill_qk_ct(1, 2) + fill_qk_ct(1, 3))
        attention(1, fill_v(8) + fill_v(9) + fill_v(10) + fill_v(11)
                  + fill_qk_ct(2, 0) + fill_qk_ct(2, 1)
                  + fill_qk_ct(2, 2) + fill_qk_ct(2, 3))
        attention(2, fill_v(12) + fill_v(13) + fill_v(14) + fill_v(15)
                  + fill_qk_ct(3, 0) + fill_qk_ct(3, 1)
                  + fill_qk_ct(3, 2) + fill_qk_ct(3, 3)
                  + fill_op(0, 0) + fill_op(0, 1) + fill_op(0, 2) + fill_op(0, 3))
        held = []

        def hold_op3(dh):
            # kt2=0 half of op(3, nqs=0): runs during attention(3) p1 (saT pair
            # 0 rows are final); finished after the p1 normalize
            def f():
                op = psum.tile([128, 512], F32, name="ps_op", tag="op", bufs=2)
                nc.tensor.matmul(
                    op[:], saT_sb[:, 0, 3 * QB:3 * QB + 128],
                    wo_sb[:, 0, dh * 512:(dh + 1) * 512],
                    start=True, stop=False,
                    skip_group_check=True,
                )
                held.append((3 * QB, dh, op))
            return f

        attention(3, fill_op(1, 0) + fill_op(1, 1) + fill_op(1, 2) + fill_op(1, 3)
                  + fill_op(2, 0) + fill_op(2, 1) + fill_op(2, 2)
                  + [hold_op3(0), hold_op3(1)])

        # ---- endgame ----
        # op(2, nqs=3) held out of the weave: it is fully independent of the
        # final normalize, so it fills the normalize-latency window
        r23 = 2 * QB + 3 * 128
        big23 = psum.tile([128, 1024], F32, name="ps_sc", tag="sc",
                          bufs=CFG["sc_bufs"])
        oe23 = sbp.tile([128, 1024], BF16, name="t_oe2", tag="oe2", bufs=4)
        for dh in range(2):
            for kt2 in range(2):
                nc.tensor.matmul(
                    big23[:, dh * 512:(dh + 1) * 512],
                    saT_sb[:, kt2, r23:r23 + 128],
                    wo_sb[:, kt2, dh * 512:(dh + 1) * 512],
                    start=(kt2 == 0), stop=(kt2 == 1),
                    skip_group_check=True,
                )
            if dh == 0:
                nc.scalar.copy(oe23[:, 0:512], big23[:, 0:512])
            else:
                nc.vector.tensor_copy(oe23[:, 512:1024], big23[:, 512:1024])
        nc.sync.dma_start(out[r23:r23 + 128, :], oe23[:])

        # block-3 pieces: kt2=0 during the normalize window on the retired
        # sc/acc banks, then kt2=1 + copies (Act||DVE) + one DMA per row block
        pieces = list(held)
        for nqs in range(1, 4):
            r0 = 3 * QB + nqs * 128
            if nqs < 3:
                big = psum.tile([128, 1024], F32, name="ps_sc", tag="sc",
                                bufs=CFG["sc_bufs"])
                ops = [big[:, 0:512], big[:, 512:1024]]
            else:
                ops = [psum.tile([128, 512], F32, name="ps_acc", tag="acc",
                                 bufs=2) for _ in range(2)]
            for dh in range(2):
                nc.tensor.matmul(
                    ops[dh], saT_sb[:, 0, r0:r0 + 128],
                    wo_sb[:, 0, dh * 512:(dh + 1) * 512],
                    start=True, stop=False,
                    skip_group_check=True,
                )
                pieces.append((r0, dh, ops[dh]))
        for r0, dh, op in pieces:
            nc.tensor.matmul(
                op, saT_sb[:, 1, r0:r0 + 128],
                wo_sb[:, 1, dh * 512:(dh + 1) * 512],
                start=False, stop=True,
                skip_group_check=True,
            )
        oes = {}
        for r0, dh, op in pieces:
            if r0 not in oes:
                oes[r0] = sbp.tile([128, 1024], BF16, name="t_oe2", tag="oe2",
                                   bufs=4)
            oe = oes[r0]
            if dh == 0:
                nc.scalar.copy(oe[:, 0:512], op)
            else:
                nc.vector.tensor_copy(oe[:, 512:1024], op)
                nc.sync.dma_start(out[r0:r0 + 128, :], oe[:])

def build(repeat=1, debug=False):
    nc = bacc.Bacc("TRN2", target_bir_lowering=False, debug=False,
                   num_devices=N_CORES)
    xT = nc.dram_tensor("xT", [D, N], BF16, kind="ExternalInput").ap()
    wqk = nc.dram_tensor("wqk", [D, 512], BF16, kind="ExternalInput").ap()
    wv = nc.dram_tensor("wv", [D, 256], BF16, kind="ExternalInput").ap()
    bqk = nc.dram_tensor("bqk", [4, 128], F32, kind="ExternalInput").ap()
    bv = nc.dram_tensor("bv", [1, 256], F32, kind="ExternalInput").ap()
    wo = nc.dram_tensor("wo", [256, 1024], F32R, kind="ExternalInput").ap()
    tri2 = nc.dram_tensor("tri2", [128, 256], F32R, kind="ExternalInput").ap()
    out = nc.dram_tensor("out", [N, D], BF16, kind="ExternalOutput").ap()
    dbg = None
    if debug:
        dbg = {
            "saT": nc.dram_tensor("dbg_saT", [256, N], F32, kind="ExternalOutput").ap(),
            "qkT": nc.dram_tensor("dbg_qkT", [512, N], F32, kind="ExternalOutput").ap(),
            "v65": nc.dram_tensor("dbg_v65", [128, NT * LH * 65], F32, kind="ExternalOutput").ap(),
        }

    with tile.TileContext(nc) as tc:
        with ExitStack() as ctx:
            _emit(nc, tc, ctx, (xT, wqk, wv, bqk, bv, wo, tri2, out), repeat=repeat,
                  dbg=dbg)
    nc.compile()
    return nc


def make_in_maps(x, Wqkv, bqkv, Wo):
    """Host-side sharding: per-core input dicts."""
    x = np.asarray(x, dtype=np.float32)
    Wqkv = np.asarray(Wqkv, dtype=np.float32)
    bqkv = np.asarray(bqkv, dtype=np.float32)
    Wo = np.asarray(Wo, dtype=np.float32)
    tri2 = np.concatenate(
        [np.zeros((128, 128), dtype=np.float32),
         np.triu(np.ones((128, 128), dtype=np.float32))], axis=1)
    in_maps = []
    for c in range(N_CORES):
        b, g = divmod(c, 4)
        hs = [4 * g + i for i in range(LH)]
        # source chunk order in Wqkv[h] columns: k (0:64), q (64:128), v (128:192)
        wqk_cols = []
        bqk_rows = []
        for p in range(2):
            hA, hB = hs[2 * p], hs[2 * p + 1]
            wqk_cols += [Wqkv[hA][:, 0:64], Wqkv[hB][:, 0:64]]    # k pair tile
            bqk_rows.append(np.concatenate([bqkv[hA][0:64], bqkv[hB][0:64]]))
            wqk_cols += [Wqkv[hA][:, 64:128], Wqkv[hB][:, 64:128]]  # q pair tile
            bqk_rows.append(np.concatenate([bqkv[hA][64:128], bqkv[hB][64:128]]))
        in_maps.append({
            "xT": np.ascontiguousarray(x[b].T).astype(ml_dtypes.bfloat16),
            "wqk": np.ascontiguousarray(
                np.concatenate(wqk_cols, axis=1)).astype(ml_dtypes.bfloat16),
            "wv": np.ascontiguousarray(
                np.concatenate([Wqkv[h][:, 128:192] for h in hs],
                               axis=1)).astype(ml_dtypes.bfloat16),
            "bqk": np.ascontiguousarray(np.stack(bqk_rows)),
            "bv": np.ascontiguousarray(
                np.concatenate([bqkv[h][128:192] for h in hs])[None, :]),
            "wo": np.ascontiguousarray(
                np.concatenate([Wo[h * HD:(h + 1) * HD, :] for h in hs], axis=0)),
            "tri2": tri2,
        })
    return in_maps


def kernel(x, Wqkv, bqkv, Wo, bo):
    if "nc" not in _CACHE:
        _CACHE["nc"] = build()
    nc = _CACHE["nc"]
    in_maps = make_in_maps(x, Wqkv, bqkv, Wo)
    res = bass_utils.run_bass_kernel_spmd(
        nc, in_maps, core_ids=list(range(N_CORES)))
    bo = np.asarray(bo, dtype=np.float32)
    full = np.empty((B, N, D), dtype=np.float32)
    for b in range(B):
        acc = res.results[4 * b]["out"].astype(np.float32).copy()
        for g in range(1, 4):
            acc += res.results[4 * b + g]["out"]
        full[b] = acc + bo[None, :]
    return full


# revision 40
# speedup vs baseline: 1.0037x; 1.0037x over previous
"""Causal self-attention (B=2, N=2048, D=1024, H=16, hd=64) on 8 trn2 NeuronCores.

Sharding: core c handles batch b = c//4 and 4 heads hs = [4*(c%4) .. 4*(c%4)+3]
(tensor-parallel over heads x data-parallel over batch). Each core computes its
heads' attention and a row-parallel partial of the output projection
(partial[n, :] = sum_{local heads} sa_h[n, :] @ Wo[h*64:(h+1)*64, :]); the host
sums the 4 partials per batch and adds the output bias.

Device schedule (v3, ~132us cost-model vs 164us baseline):
  - x/Wqkv inputs stream in bf16 (halves DMA bytes; rel err ~4e-3 vs 2e-2
    gate); q/k/v, scores and attn@v stay fp32r. Output partials in bf16.
  - One DMA instruction per tensor half (HWDGE costs ~625ns per instruction,
    so few large DMAs; wv/xT block-0 halves land first and gate the first
    matmuls at ~4us).
  - 8 dummy rank-1 matmuls ramp the PE p-state (0.65->2.4GHz needs 3us of
    continuous execution) while the first input DMAs are in flight.
  - Startup computes v(0..3)+qk(0) with per-kt partial accumulation in 8
    concurrent PSUM banks, emission ordered to match DMA arrival.
  - attention(J) emits per-(pair, key-tile) units: 2 score matmuls (head pair
    stacked on partition halves, K=64 each), one fused Exp on Act
    (scale=1/8 folded), causal tri-mask on DVE for diagonal tiles, and the
    attn@v accumulation lagging 3 units so the Exp latency hides behind later
    score matmuls. The d=3 diagonal matmuls run 256 wide (fp32r below 256
    costs 4x) with a [zeros|tri] mask zeroing the extra columns.
  - v65 tiles carry a ones-column so attn@v row 64 accumulates the softmax
    denominator for free; normalization = DVE reciprocal -> gpsimd partition
    broadcast -> DVE multiply into saT.
  - v/qk/out-projection "filler" pieces are deliberately NOT woven between
    attention units (that starves the Act pipeline); they clump at pair-flush
    and between-attention boundaries where the PE has genuine dead time
    (fpad=1000 disables mid-unit spreading).
  - Endgame: block-3 out-projection pieces spread over all 8 PSUM banks
    (sc/acc tags are retired by then); their kt2=0 halves run inside the
    final normalize latency window; copies split across Act and DVE; one
    output DMA per 128-row block.
  - PSUM bank discipline: one accumulation group per 2KB bank (start=True
    re-zeroes the bank-granular zero region, so groups must not share banks).
"""

import numpy as np
import ml_dtypes
from contextlib import ExitStack

import concourse.bass as bass
import concourse.tile as tile
from concourse import bacc, mybir
from concourse import bass_utils

F32 = mybir.dt.float32
F32R = mybir.dt.float32r
BF16 = mybir.dt.bfloat16
EXP = mybir.ActivationFunctionType.Exp

B, N, D, H, HD = 2, 2048, 1024, 16, 64
N_CORES = 8
LH = 4            # local heads per core
KT = D // 128     # 8 contraction k-tiles
NT = N // 128     # 16 n-tiles
NB = N // 512     # 4 n-blocks / q-blocks
QB = 512

_CACHE: dict = {}

CFG = {
    "at_bufs": 6,
    "oe_bufs": 4,
    "sc_bufs": 2,
    "acc_lag": 3,
    "act_primer": True,
    "n_warmup": 8,
    "fpad": 1000,
    "mask_pool": False,
    "flush_fill": 2,
    "exp_strided": False,
    "dump_split": False,
    "split_all_norm": True,
}


def _emit(nc, tc, ctx, io, repeat=1, dbg=None):
    xT, wqk, wv, bqk, bv, wo, tri2, out = io

    persist = ctx.enter_context(tc.tile_pool(name="persist", bufs=1))
    sbp = ctx.enter_context(tc.tile_pool(name="work", bufs=1))
    psum = ctx.enter_context(tc.tile_pool(name="psum", bufs=1, space="PSUM"))

    # ---- persistent SBUF tensors ----
    xT_sb = persist.tile([128, KT, N], BF16)
    wqk_sb = persist.tile([128, KT, 512], BF16)
    wv_sb = persist.tile([128, KT, 256], BF16)
    wo_sb = persist.tile([128, 2, 1024], F32R)
    bqk_sb = persist.tile([128, 4], F32)
    bv_sb = persist.tile([1, 256], F32)
    bvbc_sb = persist.tile([128, 256], F32)
    ones_sb = persist.tile([1, 128], F32)
    tri2_sb = persist.tile([128, 256], F32R)
    qkT_sb = persist.tile([128, 4, N], F32R)
    v65_sb = persist.tile([128, NT, LH * 65], F32R)
    saT_sb = persist.tile([128, 2, N], F32R)

    # ---- PE warmup + constants first (independent of any DMA) ----
    nc.vector.memset(ones_sb[:], 1.0)
    if CFG["n_warmup"]:
        # ramp the PE p-state while the first input chunks stream in: dummy
        # rank-1 matmuls on a zeroed row into a never-read psum bank
        zz = persist.tile([1, 512], F32R)
        nc.vector.memset(zz[:].bitcast(F32), 0.0)
        wps = psum.tile([128, 512], F32, name="ps_op", tag="op", bufs=2)
        for i in range(CFG["n_warmup"]):
            nc.tensor.matmul(
                wps[:], ones_sb[0:1, 0:128].bitcast(F32R), zz[:],
                start=(i == 0), stop=(i == CFG["n_warmup"] - 1),
                skip_group_check=True,
            )
    # the +1 denominator row of v65: free index h*65+64 per (nt, h)
    ones_col = v65_sb[:, :, :].rearrange("p n (h c) -> p n h c", c=65)[:, :, :, 64:65]
    nc.vector.memset(ones_col.bitcast(F32), 1.0)
    if CFG["act_primer"]:
        primer = sbp.tile([1, 1], F32, name="t_primer", tag="primer", bufs=1)
        nc.scalar.activation(primer[:], ones_sb[0:1, 0:1], EXP)

    # ---- input DMAs (SP issue order == DMA service order; HWDGE costs
    # ~625ns per instruction, so one instruction per tensor block) ----
    for h in range(2):
        k0, k1 = h * 512, (h + 1) * 512
        nc.sync.dma_start(
            wv_sb[:, 4 * h:4 * h + 4, :],
            wv[k0:k1, :].rearrange("(kt p) c -> p kt c", p=128))
        nc.sync.dma_start(
            xT_sb[:, 4 * h:4 * h + 4, 0:QB],
            xT[k0:k1, 0:QB].rearrange("(kt p) c -> p kt c", p=128))
    for h in range(2):
        k0, k1 = h * 512, (h + 1) * 512
        nc.sync.dma_start(
            wqk_sb[:, 4 * h:4 * h + 4, :],
            wqk[k0:k1, :].rearrange("(kt p) c -> p kt c", p=128))
    nc.sync.dma_start(bqk_sb[:], bqk.rearrange("t p -> p t"))
    nc.sync.dma_start(bv_sb[:], bv[:])
    nc.sync.dma_start(tri2_sb[:], tri2[:])
    for nb in range(1, NB):
        nc.sync.dma_start(
            xT_sb[:, :, nb * QB:(nb + 1) * QB],
            xT[:, nb * QB:(nb + 1) * QB].rearrange("(kt p) c -> p kt c", p=128),
        )
    for kt2 in range(2):
        nc.sync.dma_start(wo_sb[:, kt2, :], wo[kt2 * 128:(kt2 + 1) * 128, :])
    # broadcast v-bias across partitions once (Pool is idle)
    nc.gpsimd.partition_broadcast(bvbc_sb[:], bv_sb[:])

    # ---- phase pieces ----
    def v_mm(ps, nt, kt):
        # one 256-wide v matmul; a psum bank holds exactly one group
        # (start=True re-zeroes the whole 2KB zero region, so banks can't be
        # shared between accumulation groups)
        nc.tensor.matmul(
            ps[:, 0:256],
            xT_sb[:, kt, nt * 128:(nt + 1) * 128],
            wv_sb[:, kt, :],
            start=(kt == 0), stop=(kt == KT - 1),
            skip_group_check=True,
        )

    def v_drain(ps, nt):
        # psum -> v65 with the v-bias folded in (bias varies along free dim)
        dst = v65_sb[:, nt, :].rearrange("p (h c) -> p h c", c=65)[:, :, 0:64]
        src = ps[:, 0:256].rearrange("p (h c) -> p h c", c=64)
        bia = bvbc_sb[:, :].rearrange("p (h c) -> p h c", c=64)
        nc.vector.tensor_add(dst, src, bia)

    def qk_ct_mm(ps, nb, ct, kts):
        for kt in kts:
            nc.tensor.matmul(
                ps[:], wqk_sb[:, kt, ct * 128:(ct + 1) * 128],
                xT_sb[:, kt, nb * QB:(nb + 1) * QB],
                start=(kt == 0), stop=(kt == KT - 1),
                skip_group_check=True,
            )

    def qk_ct_drain(ps, nb, ct, eng="vector"):
        if eng == "scalar":
            nc.scalar.activation(
                qkT_sb[:, ct, nb * QB:(nb + 1) * QB], ps[:],
                mybir.ActivationFunctionType.Identity,
                bias=bqk_sb[:, ct:ct + 1])
        else:
            nc.vector.tensor_scalar_add(
                qkT_sb[:, ct, nb * QB:(nb + 1) * QB], ps[:], bqk_sb[:, ct:ct + 1])

    # steady-state filler pieces (each ~0.4-1 us of PE work)
    def fill_v(nt):
        ps = psum.tile([128, 512], F32, name="ps_op", tag="op", bufs=2)
        def h1():
            for kt in range(4):
                v_mm(ps, nt, kt)
        def h2():
            for kt in range(4, KT):
                v_mm(ps, nt, kt)
            v_drain(ps, nt)
        return [h1, h2]

    def fill_qk_ct(nb, ct):
        ps = psum.tile([128, 512], F32, name="ps_op", tag="op", bufs=2)
        def h1():
            qk_ct_mm(ps, nb, ct, range(4))
        def h2():
            qk_ct_mm(ps, nb, ct, range(4, KT))
            # Act has slack during att(0)/att(1); DVE is the tighter engine
            qk_ct_drain(ps, nb, ct, eng="scalar" if nb <= 2 else "vector")
        return [h1, h2]

    def fill_op(J, nqs, copy_eng="vector", dma_per_dh=False):
        # one output-projection row-block piece: both dh halves -> one DMA
        r0 = J * QB + nqs * 128
        oe = sbp.tile([128, 1024], BF16, name="t_oe", tag="oe", bufs=CFG["oe_bufs"])
        def piece(dh):
            def f():
                op = psum.tile([128, 512], F32, name="ps_op", tag="op", bufs=2)
                for kt2 in range(2):
                    nc.tensor.matmul(
                        op[:], saT_sb[:, kt2, r0:r0 + 128],
                        wo_sb[:, kt2, dh * 512:(dh + 1) * 512],
                        start=(kt2 == 0), stop=(kt2 == 1),
                        skip_group_check=True,
                    )
                if copy_eng == "scalar":
                    nc.scalar.copy(oe[:, dh * 512:(dh + 1) * 512], op[:])
                else:
                    nc.vector.tensor_copy(oe[:, dh * 512:(dh + 1) * 512], op[:])
                if dma_per_dh:
                    nc.sync.dma_start(out[r0:r0 + 128, dh * 512:(dh + 1) * 512],
                                      oe[:, dh * 512:(dh + 1) * 512])
                elif dh == 1:
                    nc.sync.dma_start(out[r0:r0 + 128, :], oe[:])
            return f
        return [piece(0), piece(1)]

    # ---- attention ----
    def emit_acc(J, accs, unit, n_t):
        t, p, at, c0e = unit
        for s in range(2):
            nc.tensor.matmul(
                accs[p][s][0:65, c0e:512],
                v65_sb[:, t, (2 * p + s) * 65:(2 * p + s) * 65 + 65],
                at[:, s * 512 + c0e:(s + 1) * 512],
                start=(t == 0), stop=(t == n_t - 1),
                skip_group_check=True,
            )

    def normalize(J, accs, p, split=False):
        halves = ((0, 256), (256, 512)) if split else ((0, 512),)
        for a, b in halves:
            bcs = []
            for s in range(2):
                rc = sbp.tile([1, 512], F32, name="t_rc", tag="rc", bufs=4)
                nc.vector.reciprocal(rc[:, 0:b - a], accs[p][s][64:65, a:b])
                bc = sbp.tile([64, 512], F32, name="t_bc", tag="bc", bufs=4)
                nc.gpsimd.partition_broadcast(bc[:, 0:b - a], rc[:, 0:b - a])
                bcs.append(bc)
            for s in range(2):
                nc.vector.tensor_mul(
                    saT_sb[s * 64:(s + 1) * 64, p, J * QB + a:J * QB + b],
                    accs[p][s][0:64, a:b], bcs[s][:, 0:b - a],
                )

    def attention(J, fillers):
        n_t = 4 * J + 4
        n_units = n_t * 2
        n_fill = len(fillers)
        fill_i = 0
        unit_i = 0
        for p in range(2):
            accs = {p: [psum.tile([128, 512], F32, name="ps_acc", tag="acc",
                                  bufs=2) for _ in range(2)]}
            pend = []
            for t in range(n_t):
                d = t - 4 * J
                c0 = max(d, 0) * 128
                c0e = min(c0, 256)
                sc = psum.tile([128, 1024], F32, name="ps_sc", tag="sc",
                               bufs=CFG["sc_bufs"])
                for s in range(2):
                    nc.tensor.matmul(
                        sc[:, s * 512 + c0e:(s + 1) * 512],
                        qkT_sb[s * 64:(s + 1) * 64, 2 * p, t * 128:(t + 1) * 128],
                        qkT_sb[s * 64:(s + 1) * 64, 2 * p + 1, J * QB + c0e:(J + 1) * QB],
                        start=True, stop=True,
                    )
                at = sbp.tile([128, 1024], F32R, name="t_at", tag="at",
                              bufs=CFG["at_bufs"])
                if CFG["exp_strided"] and c0e > 0:
                    atv3 = at[:, :].rearrange(
                        "p (s w) -> p s w", s=2)[:, :, c0e:512]
                    scv3 = sc[:, :].rearrange(
                        "p (s w) -> p s w", s=2)[:, :, c0e:512]
                    nc.scalar.activation(atv3, scv3, EXP, scale=0.125)
                else:
                    nc.scalar.activation(at[:, c0e:1024], sc[:, c0e:1024],
                                         EXP, scale=0.125)
                mul_eng = nc.gpsimd if CFG["mask_pool"] else nc.vector
                if d == 3:
                    atv = at[:, 256:1024].rearrange(
                        "p (s c) -> p s c", c=256)[:, ::2, :]
                    mul_eng.tensor_mul(
                        atv, atv, tri2_sb[:, None, :].broadcast_to([128, 2, 256]))
                elif d >= 0:
                    atv = at[:, c0:c0 + 640].rearrange(
                        "p (s c) -> p s c", c=128)[:, ::4, :]
                    mul_eng.tensor_mul(
                        atv, atv,
                        tri2_sb[:, None, 128:256].broadcast_to([128, 2, 128]))
                pend.append((t, p, at, c0e))
                if len(pend) > CFG["acc_lag"]:
                    emit_acc(J, accs, pend.pop(0), n_t)
                unit_i += 1
                # proportional filler spreading (padded so some remain for the
                # pair-flush stalls)
                while fill_i < n_fill and fill_i * (n_units + CFG["fpad"]) < unit_i * n_fill:
                    fillers[fill_i]()
                    fill_i += 1
            # flush the lag: earlier accs are already unblocked - emit them
            # first, then cover the last Exp's latency with fillers
            while pend:
                emit_acc(J, accs, pend.pop(0), n_t)
                if len(pend) == 1:
                    for _ in range(CFG["flush_fill"]):
                        if fill_i < n_fill:
                            fillers[fill_i]()
                            fill_i += 1
            normalize(J, accs, p, split=(CFG["split_all_norm"] or (J == 3 and p == 1)))
            # dump a share of the remaining fillers at this pair boundary
            if CFG["dump_split"] and p == 0:
                target = fill_i + (n_fill - fill_i) // 2
                while fill_i < target:
                    fillers[fill_i]()
                    fill_i += 1
        while fill_i < n_fill:
            fillers[fill_i]()
            fill_i += 1

    # ---- emission ----
    for _rep in range(repeat):
        # startup: kt-partial v(0..3) + qk(0), 8 concurrent psum banks
        v_ps = ([psum.tile([128, 512], F32, name="ps_acc", tag="acc", bufs=2)
                 for _ in range(2)]
                + [psum.tile([128, 512], F32, name="ps_op", tag="op", bufs=2)
                   for _ in range(2)])
        qk_ps = [psum.tile([128, 1024], F32, name="ps_sc", tag="sc",
                           bufs=CFG["sc_bufs"]) for _ in range(2)]
        # emission matches DMA arrival: wv/xT halves land before wqk halves
        for kh in range(2):
            for kt in range(4 * kh, 4 * kh + 4):
                for i in range(4):
                    v_mm(v_ps[i], i, kt)
        for kh in range(2):
            for kt in range(4 * kh, 4 * kh + 4):
                for ct in range(4):
                    qk_ct_mm(qk_ps[ct // 2][:, (ct % 2) * 512:(ct % 2 + 1) * 512],
                             0, ct, [kt])
        for ct in range(4):
            qk_ct_drain(qk_ps[ct // 2][:, (ct % 2) * 512:(ct % 2 + 1) * 512],
                        0, ct, eng="scalar")
        for i in range(4):
            v_drain(v_ps[i], i)

        attention(0, fill_v(4) + fill_v(5) + fill_v(6) + fill_v(7)
                  + fill_qk_ct(1, 0) + fill_qk_ct(1, 1)
                  + fill_qk_ct(1, 2) + fill_qk_ct(1, 3))
        attention(1, fill_v(8) + fill_v(9) + fill_v(10) + fill_v(11)
                  + fill_qk_ct(2, 0) + fill_qk_ct(2, 1)
                  + fill_qk_ct(2, 2) + fill_qk_ct(2, 3))
        attention(2, fill_v(12) + fill_v(13) + fill_v(14) + fill_v(15)
                  + fill_qk_ct(3, 0) + fill_qk_ct(3, 1)
                  + fill_qk_ct(3, 2) + fill_qk_ct(3, 3)
                  + fill_op(0, 0) + fill_op(0, 1) + fill_op(0, 2) + fill_op(0, 3))
        held = []

        def hold_op3(dh):
            # kt2=0 half of op(3, nqs=0): runs during attention(3) p1 (saT pair
            # 0 rows are final); finished after the p1 normalize
            def f():
                op = psum.tile([128, 512], F32, name="ps_op", tag="op", bufs=2)
                nc.tensor.matmul(
                    op[:], saT_sb[:, 0, 3 * QB:3 * QB + 128],
                    wo_sb[:, 0, dh * 512:(dh + 1) * 512],
                    start=True, stop=False,
                    skip_group_check=True,
                )
                held.append((3 * QB, dh, op))
            return f

        attention(3, fill_op(1, 0) + fill_op(1, 1) + fill_op(1, 2) + fill_op(1, 3)
                  + fill_op(2, 0) + fill_op(2, 1) + fill_op(2, 2)
                  + [hold_op3(0), hold_op3(1)])

        # ---- endgame ----
        # op(2, nqs=3) held out of the weave: it is fully independent of the
        # final normalize, so it fills the normalize-latency window
        r23 = 2 * QB + 3 * 128
        big23 = psum.tile([128, 1024], F32, name="ps_sc", tag="sc",
                          bufs=CFG["sc_bufs"])
        oe23 = sbp.tile([128, 1024], BF16, name="t_oe2", tag="oe2", bufs=4)
        for dh in range(2):
            for kt2 in range(2):
                nc.tensor.matmul(
                    big23[:, dh * 512:(dh + 1) * 512],
                    saT_sb[:, kt2, r23:r23 + 128],
                    wo_sb[:, kt2, dh * 512:(dh + 1) * 512],
                    start=(kt2 == 0), stop=(kt2 == 1),
                    skip_group_check=True,
                )
            if dh == 0:
                nc.scalar.copy(oe23[:, 0:512], big23[:, 0:512])
            else:
                nc.vector.tensor_copy(oe23[:, 512:1024], big23[:, 512:1024])
        nc.sync.dma_start(out[r23:r23 + 128, :], oe23[:])

        # block-3 pieces: kt2=0 during the normalize window on the retired
        # sc/acc banks, then kt2=1 + copies (Act||DVE) + one DMA per row block
        pieces = list(held)
        for nqs in range(1, 4):
            r0 = 3 * QB + nqs * 128
            if nqs < 3:
                big = psum.tile([128, 1024], F32, name="ps_sc", tag="sc",
                                bufs=CFG["sc_bufs"])
                ops = [big[:, 0:512], big[:, 512:1024]]
            else:
                ops = [psum.tile([128, 512], F32, name="ps_acc", tag="acc",
                                 bufs=2) for _ in range(2)]
            for dh in range(2):
                nc.tensor.matmul(
                    ops[dh], saT_sb[:, 0, r0:r0 + 128],
                    wo_sb[:, 0, dh * 512:(dh + 1) * 512],
                    start=True, stop=False,
                    skip_group_check=True,
                )
                pieces.append((r0, dh, ops[dh]))
        for r0, dh, op in pieces:
            nc.tensor.matmul(
                op, saT_sb[:, 1, r0:r0 + 128],
                wo_sb[:, 1, dh * 512:(dh + 1) * 512],
                start=False, stop=True,
                skip_group_check=True,
            )
        oes = {}
        for r0, dh, op in pieces:
            if r0 not in oes:
                oes[r0] = sbp.tile([128, 1024], BF16, name="t_oe2", tag="oe2",
                                   bufs=4)
            oe = oes[r0]
            if dh == 0:
                nc.scalar.copy(oe[:, 0:512], op)
            else:
                nc.vector.tensor_copy(oe[:, 512:1024], op)
                nc.sync.dma_start(out[r0:r0 + 128, :], oe[:])

def build(repeat=1, debug=False):
    nc = bacc.Bacc("TRN2", target_bir_lowering=False, debug=False,
                   num_devices=N_CORES)
    xT = nc.dram_tensor("xT", [D, N], BF16, kind="ExternalInput").ap()
    wqk = nc.dram_tensor("wqk", [D, 512], BF16, kind="ExternalInput").ap()
    wv = nc.dram_tensor("wv", [D, 256], BF16, kind="ExternalInput").ap()
    bqk = nc.dram_tensor("bqk", [4, 128], F32, kind="ExternalInput").ap()
    bv = nc.dram_tensor("bv", [1, 256], F32, kind="ExternalInput").ap()
    wo = nc.dram_tensor("wo", [256, 1024], F32R, kind="ExternalInput").ap()
    tri2 = nc.dram_tensor("tri2", [128, 256], F32R, kind="ExternalInput").ap()
    out = nc.dram_tensor("out", [N, D], BF16, kind="ExternalOutput").ap()
    dbg = None
    if debug:
        dbg = {
            "saT": nc.dram_tensor("dbg_saT", [256, N], F32, kind="ExternalOutput").ap(),
            "qkT": nc.dram_tensor("dbg_qkT", [512, N], F32, kind="ExternalOutput").ap(),
            "v65": nc.dram_tensor("dbg_v65", [128, NT * LH * 65], F32, kind="ExternalOutput").ap(),
        }

    with tile.TileContext(nc) as tc:
        with ExitStack() as ctx:
            _emit(nc, tc, ctx, (xT, wqk, wv, bqk, bv, wo, tri2, out), repeat=repeat,
                  dbg=dbg)
    nc.compile()
    return nc


def make_in_maps(x, Wqkv, bqkv, Wo):
    """Host-side sharding: per-core input dicts."""
    x = np.asarray(x, dtype=np.float32)
    Wqkv = np.asarray(Wqkv, dtype=np.float32)
    bqkv = np.asarray(bqkv, dtype=np.float32)
    Wo = np.asarray(Wo, dtype=np.float32)
    tri2 = np.concatenate(
        [np.zeros((128, 128), dtype=np.float32),
         np.triu(np.ones((128, 128), dtype=np.float32))], axis=1)
    in_maps = []
    for c in range(N_CORES):
        b, g = divmod(c, 4)
        hs = [4 * g + i for i in range(LH)]
        # source chunk order in Wqkv[h] columns: k (0:64), q (64:128), v (128:192)
        wqk_cols = []
        bqk_rows = []
        for p in range(2):
            hA, hB = hs[2 * p], hs[2 * p + 1]
            wqk_cols += [Wqkv[hA][:, 0:64], Wqkv[hB][:, 0:64]]    # k pair tile
            bqk_rows.append(np.concatenate([bqkv[hA][0:64], bqkv[hB][0:64]]))
            wqk_cols += [Wqkv[hA][:, 64:128], Wqkv[hB][:, 64:128]]  # q pair tile
            bqk_rows.append(np.concatenate([bqkv[hA][64:128], bqkv[hB][64:128]]))
        in_maps.append({
            "xT": np.ascontiguousarray(x[b].T).astype(ml_dtypes.bfloat16),
            "wqk": np.ascontiguousarray(
                np.concatenate(wqk_cols, axis=1)).astype(ml_dtypes.bfloat16),
            "wv": np.ascontiguousarray(
                np.concatenate([Wqkv[h][:, 128:192] for h in hs],
                               axis=1)).astype(ml_dtypes.bfloat16),
            "bqk": np.ascontiguousarray(np.stack(bqk_rows)),
            "bv": np.ascontiguousarray(
                np.concatenate([bqkv[h][128:192] for h in hs])[None, :]),
            "wo": np.ascontiguousarray(
                np.concatenate([Wo[h * HD:(h + 1) * HD, :] for h in hs], axis=0)),
            "tri2": tri2,
        })
    return in_maps


def kernel(x, Wqkv, bqkv, Wo, bo):
    if "nc" not in _CACHE:
        _CACHE["nc"] = build()
    nc = _CACHE["nc"]
    in_maps = make_in_maps(x, Wqkv, bqkv, Wo)
    res = bass_utils.run_bass_kernel_spmd(
        nc, in_maps, core_ids=list(range(N_CORES)))
    bo = np.asarray(bo, dtype=np.float32)
    full = np.empty((B, N, D), dtype=np.float32)
    for b in range(B):
        acc = res.results[4 * b]["out"].astype(np.float32).copy()
        for g in range(1, 4):
            acc += res.results[4 * b + g]["out"]
        full[b] = acc + bo[None, :]
    return full
